# revision 1
# baseline (speedup 1.0000x reference)
"""Greedy CTC decoder on Trainium2 (Bass/Tile), sharded over 8 NeuronCores.

Input : emission [65536, 512] float32 (full, unsharded)
Output: (index [65536] int32, keep [65536] bool) matching the reference:
    index = argmax(emission, axis=-1)
    char  = index - 1 (blank 0 -> -1)
    keep  = (char != prev_char) & (char != -1)
          = (index != prev_index) & (index != 0),  prev of t=0 is a sentinel

Sharding: timestep axis T split across 8 cores (8192 rows each). Inside a
core, partition p owns the 64 consecutive timesteps p*64..p*64+63, so the
repeat-collapse comparison is a free-dim shift. The 64-step chunk boundary
(prev of j=0 lives on partition p-1) is resolved with one tiny SBUF->SBUF
DMA; the 7 shard boundaries are fixed on the host.
"""

import numpy as np

import concourse.bacc as bacc
import concourse.mybir as mybir
from concourse.tile import TileContext
from concourse.bass_utils import run_bass_kernel_spmd

N_CORES = 8
T_FULL = 65536
V = 512
P = 128
T_SHARD = T_FULL // N_CORES          # 8192
JPP = T_SHARD // P                   # 64 timesteps per partition
# chunk sizes (timesteps per partition per DMA): small first chunks so the
# DVE starts early, 2 MiB chunks later for full DMA efficiency
CHUNKS = [2, 2, 4] + [8] * 7
HALF = 32                            # keep-mask split point (after 6 chunks)
SENTINEL = 1000000.0                 # != any vocab index, exact in fp32

_prog_cache = {}


def _build():
    nc = bacc.Bacc(None, target_bir_lowering=False)

    em_h = nc.dram_tensor("emission", [T_SHARD, V], mybir.dt.float32,
                          kind="ExternalInput")
    idx_h = nc.dram_tensor("idx_out", [T_SHARD], mybir.dt.uint32,
                           kind="ExternalOutput")
    keep_h = nc.dram_tensor("keep_out", [T_SHARD], mybir.dt.uint8,
                            kind="ExternalOutput")

    # [T_SHARD, V] -> [P, JPP, V]: partition p holds rows p*JPP .. p*JPP+JPP-1
    em3 = em_h[:, :].rearrange("(p j) v -> p j v", p=P)
    idx_out2 = idx_h[:].rearrange("(p j) -> p j", p=P)
    keep_out2 = keep_h[:].rearrange("(p j) -> p j", p=P)

    with TileContext(nc) as tc:
        with (
            tc.tile_pool(name="io", bufs=4) as io_pool,
            tc.tile_pool(name="mx", bufs=4) as mx_pool,
            tc.tile_pool(name="acc", bufs=1) as acc_pool,
        ):
            # raw argmax stream-indices: for 8-row chunks one FIND_INDEX8
            # searches all 8 rows at once (needle k = row k's max), so the
            # value is (k*512 + argmax). Cross-row bitwise-equal collisions
            # are detected host-side via the k bits and repaired there.
            idxr = acc_pool.tile([P, JPP], mybir.dt.uint32)
            small8 = acc_pool.tile([P, 8, 8], mybir.dt.uint32)
            idxc = acc_pool.tile([P, JPP], mybir.dt.uint32)
            offs = acc_pool.tile([P, JPP], mybir.dt.uint32)
            offs_np = np.zeros((P, JPP), dtype=np.uint32)
            for jj in range(8, JPP):
                offs_np[:, jj] = (jj % 8) * V
            offs_dram = nc.inline_tensor(offs_np, name="offs_const")
            nc.sync.dma_start(out=offs[:, :], in_=offs_dram[:, :])
            neq = acc_pool.tile([P, JPP], mybir.dt.uint8)
            nz = acc_pool.tile([P, JPP], mybir.dt.uint8)
            keep = acc_pool.tile([P, JPP], mybir.dt.uint8)

            def keep_phase(lo, hi):
                """Repeat-collapse for columns [lo, hi) on GpSimd (DVE stays
                on max_index). Column 0 is deferred to the caller."""
                v = nc.vector
                # strip the within-chunk row offset: idxc = idxr - k*512
                v.tensor_tensor(out=idxc[:, lo:hi], in0=idxr[:, lo:hi],
                                in1=offs[:, lo:hi],
                                op=mybir.AluOpType.subtract)
                lo1 = max(lo, 1)  # column 0 needs the cross-partition prev
                v.tensor_tensor(out=neq[:, lo1:hi], in0=idxc[:, lo1:hi],
                                in1=idxc[:, lo1 - 1:hi - 1],
                                op=mybir.AluOpType.not_equal)
                v.tensor_scalar(out=nz[:, lo:hi], in0=idxc[:, lo:hi],
                                scalar1=0.0, scalar2=None,
                                op0=mybir.AluOpType.not_equal)
                v.tensor_tensor(out=keep[:, lo1:hi], in0=neq[:, lo1:hi],
                                in1=nz[:, lo1:hi], op=mybir.AluOpType.mult)
                nc.sync.dma_start(out=idx_out2[:, lo:hi], in_=idxr[:, lo:hi])
                nc.sync.dma_start(out=keep_out2[:, lo1:hi],
                                  in_=keep[:, lo1:hi])

            j = 0
            for c, n in enumerate(CHUNKS):
                tile = io_pool.tile([P, n, V], mybir.dt.float32)
                nc.sync.dma_start(out=tile[:, :, :], in_=em3[:, j:j + n, :])
                # one reduce for all n rows' maxes (552ns/row vs 608 for
                # per-row InstMax)
                rowmax = mx_pool.tile([P, 8], mybir.dt.float32)
                nc.vector.tensor_reduce(out=rowmax[:, 0:n], in_=tile[:, :, :],
                                        axis=mybir.AxisListType.X,
                                        op=mybir.AluOpType.max)
                if n == 8:
                    # one FIND_INDEX8 for all 8 rows: needles are the 8 row
                    # maxes, scanned over the whole 4096-element chunk
                    nc.vector.max_index(
                        out=idxr[:, j:j + 8],
                        in_max=rowmax[:, :],
                        in_values=tile[:, :, :].rearrange("p a v -> p (a v)"))
                else:
                    for k in range(n):
                        nc.vector.max_index(
                            out=small8[:, j + k, :],
                            in_max=rowmax[:, k:k + 1].broadcast_to((P, 8)),
                            in_values=tile[:, k, :])
                j += n
                if j == 8:
                    # compact the per-row results of the small head chunks
                    nc.vector.tensor_copy(idxr[:, 0:8], small8[:, :, 0])
                if j == HALF:
                    keep_phase(0, HALF)

            keep_phase(HALF, JPP)
            # column 0 of each partition (t % 64 == 0) is resolved on the
            # host: it needs the previous partition/shard's last index, and
            # a 128-byte cross-partition DMA costs ~3us of tail latency here

    nc.compile()
    return nc


def _get_prog():
    if "nc" not in _prog_cache:
        _prog_cache["nc"] = _build()
    return _prog_cache["nc"]


def run_sharded(emission: np.ndarray, **spmd_kwargs):
    """Run the SPMD kernel; returns (idx int32 [T], keep bool [T], results)."""
    emission = np.ascontiguousarray(np.asarray(emission, dtype=np.float32))
    assert emission.shape == (T_FULL, V), emission.shape
    nc = _get_prog()
    in_maps = [
        {"emission": np.ascontiguousarray(emission[c * T_SHARD:(c + 1) * T_SHARD])}
        for c in range(N_CORES)
    ]
    res = run_bass_kernel_spmd(nc, in_maps, list(range(N_CORES)), **spmd_kwargs)
    raw = np.concatenate([res.results[c]["idx_out"] for c in range(N_CORES)])
    keep = np.concatenate([res.results[c]["keep_out"] for c in range(N_CORES)])
    idx = (raw & (V - 1)).astype(np.int32)
    keep = keep.astype(bool, copy=False)
    # detect cross-row collisions in the batched FIND_INDEX8: the needle
    # matched in the wrong row's segment
    j_arr = np.arange(T_FULL) % JPP
    expected = np.where(j_arr < 8, 0, j_arr % 8).astype(np.uint32)
    corrupt = np.nonzero((raw >> 9) != expected)[0]
    for t in corrupt:
        idx[t] = int(np.argmax(emission[t]))
    for t0 in corrupt:
        for t in (t0, t0 + 1):
            if t < T_FULL:
                keep[t] = bool((idx[t] != (idx[t - 1] if t else -1))
                               and (idx[t] != 0))
    # boundary exchange: the device leaves every 64-step chunk's first
    # timestep unresolved (cross-partition/shard prev); fix them all here
    b = np.arange(64, T_FULL, 64)
    keep[b] = (idx[b] != idx[b - 1]) & (idx[b] != 0)
    keep[0] = idx[0] != 0
    return idx, keep, res


def kernel(emission: np.ndarray):
    idx, keep, _ = run_sharded(emission)
    return idx, keep



# revision 3
# speedup vs baseline: 1.1587x; 1.1587x over previous
"""Greedy CTC decoder on Trainium2 (Bass/Tile), sharded over 8 NeuronCores.

Input : emission [65536, 512] float32 (full, unsharded)
Output: (index [65536] int32, keep [65536] bool) matching the reference:
    index = argmax(emission, axis=-1)
    keep  = (index != prev_index) & (index != 0), prev of t=0 is a sentinel

Sharding: timestep axis T split across 8 cores (8192 rows each). Inside a
core, partition p owns the 64 consecutive timesteps p*64..p*64+63.

Device pipeline per chunk of n rows (three engines balanced under the DMA
roofline of ~47us for the 16.8 MiB shard):
  1. GpSimd : fold1  p1[:, n, 256] = max(x[:, :, 0:256], x[:, :, 256:512])
  2. DVE    : fold2  p2[:, n, 128] = max(p1[:, :, 0:128], p1[:, :, 128:256])
  3. DVE    : rowmax[:, n]         = reduce_max(p2)        (the needles)
  4. DVE    : FIND_INDEX8 over p2  -> raw = k*128 + u  (u = argmax slot)
The true argmax is one of {u, u+128, u+256, u+384}; the host resolves the
two folded bits with a vectorized 4-column compare (exact, including
first-occurrence tie-breaks) and derives the keep mask from idx.
"""

import numpy as np

import concourse.bacc as bacc
import concourse.mybir as mybir
from concourse.tile import TileContext
from concourse.bass_utils import run_bass_kernel_spmd

N_CORES = 8
T_FULL = 65536
V = 512
H1 = V // 2                          # 256, after gpsimd fold
H2 = V // 4                          # 128, after DVE fold
P = 128
T_SHARD = T_FULL // N_CORES          # 8192
JPP = T_SHARD // P                   # 64 timesteps per partition
# chunk sizes (timesteps per partition per DMA): small first chunks so
# compute starts early, 2 MiB chunks in steady state, tapered tail so the
# last chunk's fold->reduce->find chain is short
CHUNKS = [1, 1, 2, 4] + [8] * 6 + [4, 2, 1, 1]
assert sum(CHUNKS) == JPP

_prog_cache = {}


def _build():
    nc = bacc.Bacc(None, target_bir_lowering=False)

    em_h = nc.dram_tensor("emission", [T_SHARD, V], mybir.dt.float32,
                          kind="ExternalInput")
    idx_h = nc.dram_tensor("idx_out", [T_SHARD], mybir.dt.uint32,
                           kind="ExternalOutput")

    # [T_SHARD, V] -> [P, JPP, V]: partition p holds rows p*JPP .. p*JPP+JPP-1
    em3 = em_h[:, :].rearrange("(p j) v -> p j v", p=P)
    idx_out2 = idx_h[:].rearrange("(p j) -> p j", p=P)

    with TileContext(nc) as tc:
        with (
            tc.tile_pool(name="io", bufs=4) as io_pool,
            tc.tile_pool(name="fold", bufs=4) as fold_pool,
            tc.tile_pool(name="mx", bufs=4) as mx_pool,
        ):
            j = 0
            for n in CHUNKS:
                tile = io_pool.tile([P, n, V], mybir.dt.float32)
                nc.sync.dma_start(out=tile[:, :, :], in_=em3[:, j:j + n, :])
                # fold1: vocab 512 -> 256 (TT reads 2 elems/cycle on DVE)
                p1 = fold_pool.tile([P, n, H1], mybir.dt.float32)
                nc.vector.tensor_tensor(out=p1[:, :, :],
                                        in0=tile[:, :, 0:H1],
                                        in1=tile[:, :, H1:V],
                                        op=mybir.AluOpType.max)
                # fold2 on DVE: 256 -> 128 (TT reads 2 elems/cycle)
                p2 = fold_pool.tile([P, n, H2], mybir.dt.float32)
                nc.vector.tensor_tensor(out=p2[:, :, :],
                                        in0=p1[:, :, 0:H2],
                                        in1=p1[:, :, H2:H1],
                                        op=mybir.AluOpType.max)
                # per-row maxes (the needles)
                rowmax = mx_pool.tile([P, 8], mybir.dt.float32)
                nc.vector.tensor_reduce(out=rowmax[:, 0:n], in_=p2[:, :, :],
                                        axis=mybir.AxisListType.X,
                                        op=mybir.AluOpType.max)
                # one FIND_INDEX8 for up to 8 rows: needle k = row k's max,
                # scanned over the folded chunk; raw value = k*128 + u.
                # Unused needle slots hold stale floats; their outputs are
                # ignored host-side.
                idxr = mx_pool.tile([P, 8], mybir.dt.uint32)
                nc.vector.max_index(
                    out=idxr[:, :],
                    in_max=rowmax[:, :],
                    in_values=p2[:, :, :].rearrange("p a v -> p (a v)"))
                nc.sync.dma_start(out=idx_out2[:, j:j + n], in_=idxr[:, 0:n])
                j += n

    nc.compile()
    return nc


def _get_prog():
    if "nc" not in _prog_cache:
        _prog_cache["nc"] = _build()
    return _prog_cache["nc"]


# map j (timestep-within-partition) -> its chunk's first j (for collision
# detection: raw >> 7 must equal j - j0)
_J0 = np.empty(JPP, dtype=np.uint32)
_j = 0
for _n in CHUNKS:
    _J0[_j:_j + _n] = _j
    _j += _n


def run_sharded(emission: np.ndarray, **spmd_kwargs):
    """Run the SPMD kernel; returns (idx int32 [T], keep bool [T], results)."""
    emission = np.ascontiguousarray(np.asarray(emission, dtype=np.float32))
    assert emission.shape == (T_FULL, V), emission.shape
    nc = _get_prog()
    in_maps = [
        {"emission": np.ascontiguousarray(emission[c * T_SHARD:(c + 1) * T_SHARD])}
        for c in range(N_CORES)
    ]
    res = run_bass_kernel_spmd(nc, in_maps, list(range(N_CORES)), **spmd_kwargs)
    raw = np.concatenate([res.results[c]["idx_out"] for c in range(N_CORES)])

    t_all = np.arange(T_FULL)
    j_arr = (t_all % JPP).astype(np.uint32)
    u = (raw & np.uint32(H2 - 1)).astype(np.int64)          # slot in [0,128)
    kk = raw >> np.uint32(7)                                # row-in-chunk bits
    expected = j_arr - _J0[j_arr]
    corrupt = np.nonzero(kk != expected)[0]

    # resolve the two folded bits: candidates u, u+128, u+256, u+384
    cand = np.stack([emission[t_all, u + k * H2] for k in range(4)], axis=1)
    bits = np.argmax(cand, axis=1)                          # first max wins
    idx = (u + bits * H2).astype(np.int32)

    # cross-row FIND_INDEX8 collisions (needle matched an earlier row's
    # segment): recompute those rows exactly
    for t in corrupt:
        idx[t] = int(np.argmax(emission[t]))

    keep = np.empty(T_FULL, dtype=bool)
    keep[0] = idx[0] != 0
    keep[1:] = (idx[1:] != idx[:-1]) & (idx[1:] != 0)
    return idx, keep, res


def kernel(emission: np.ndarray):
    idx, keep, _ = run_sharded(emission)
    return idx, keep


# revision 4
# speedup vs baseline: 1.2326x; 1.0637x over previous
"""Greedy CTC decoder on Trainium2 (Bass/Tile), sharded over 8 NeuronCores.

Input : emission [65536, 512] float32 (full, unsharded)
Output: (index [65536] int32, keep [65536] bool) matching the reference:
    index = argmax(emission, axis=-1)
    keep  = (index != prev_index) & (index != 0), prev of t=0 is a sentinel

Sharding: timestep axis T split across 8 cores (8192 rows each). Inside a
core, partition p owns the 64 consecutive timesteps p*64..p*64+63.

Device pipeline per chunk of n rows:
  1. DVE fold1  p1[:, n, 256] = max(x[:, :, 0:256], x[:, :, 256:512])
  2. DVE fold2  p2[:, n, 128] = max(p1[:, :, 0:128], p1[:, :, 128:256])
  3. DVE rowmax[:, n]         = reduce_max(p2)        (the needles)
  4. DVE FIND_INDEX8 over p2  -> raw = k*128 + u  (u = argmax slot)
TT folds read 2 elems/cycle, reduce+find read 1, so the argmax costs ~1.5
data passes on the DVE instead of 2. All finds accumulate into one SBUF
tile, stored with a single DMA at the end. The true argmax is one of
{u, u+128, u+256, u+384}; the host resolves the two folded bits with a
vectorized 4-column compare (exact, including first-occurrence
tie-breaks) and derives the keep mask from idx.
"""

import numpy as np

import concourse.bacc as bacc
import concourse.mybir as mybir
from concourse.tile import TileContext
from concourse.bass_utils import run_bass_kernel_spmd

N_CORES = 8
T_FULL = 65536
V = 512
H1 = V // 2                          # 256, after fold1
H2 = V // 4                          # 128, after fold2
P = 128
T_SHARD = T_FULL // N_CORES          # 8192
JPP = T_SHARD // P                   # 64 timesteps per partition
# chunk sizes (timesteps per partition per DMA): small first chunks so
# compute starts early, 2 MiB chunks in steady state, tapered tail so the
# last chunk's fold->reduce->find chain is short
CHUNKS = [1, 1, 2, 4] + [8] * 6 + [4, 2, 1, 1]
NCH = len(CHUNKS)
assert sum(CHUNKS) == JPP

_prog_cache = {}


def _build():
    nc = bacc.Bacc(None, target_bir_lowering=False)

    em_h = nc.dram_tensor("emission", [T_SHARD, V], mybir.dt.float32,
                          kind="ExternalInput")
    # padded find results: slot [p, c, k] = chunk c needle k on partition p
    idx_h = nc.dram_tensor("idx_out", [P, NCH, 8], mybir.dt.uint32,
                           kind="ExternalOutput")

    # [T_SHARD, V] -> [P, JPP, V]: partition p holds rows p*JPP .. p*JPP+JPP-1
    em3 = em_h[:, :].rearrange("(p j) v -> p j v", p=P)

    with TileContext(nc) as tc:
        with (
            tc.tile_pool(name="io", bufs=8) as io_pool,
            tc.tile_pool(name="fold", bufs=3) as fold_pool,
            tc.tile_pool(name="mx", bufs=4) as mx_pool,
            tc.tile_pool(name="acc", bufs=1) as acc_pool,
        ):
            idxacc = acc_pool.tile([P, NCH, 8], mybir.dt.uint32)
            j = 0
            for c, n in enumerate(CHUNKS):
                tile = io_pool.tile([P, n, V], mybir.dt.float32)
                nc.sync.dma_start(out=tile[:, :, :], in_=em3[:, j:j + n, :])
                # fold1: vocab 512 -> 256 (TT reads 2 elems/cycle)
                p1 = fold_pool.tile([P, n, H1], mybir.dt.float32)
                nc.vector.tensor_tensor(out=p1[:, :, :],
                                        in0=tile[:, :, 0:H1],
                                        in1=tile[:, :, H1:V],
                                        op=mybir.AluOpType.max)
                # fold2: 256 -> 128
                p2 = fold_pool.tile([P, n, H2], mybir.dt.float32)
                nc.vector.tensor_tensor(out=p2[:, :, :],
                                        in0=p1[:, :, 0:H2],
                                        in1=p1[:, :, H2:H1],
                                        op=mybir.AluOpType.max)
                # per-row maxes (the needles)
                rowmax = mx_pool.tile([P, 8], mybir.dt.float32)
                nc.vector.tensor_reduce(out=rowmax[:, 0:n], in_=p2[:, :, :],
                                        axis=mybir.AxisListType.X,
                                        op=mybir.AluOpType.max)
                # one FIND_INDEX8 for up to 8 rows: needle k = row k's max,
                # scanned over the folded chunk; raw value = k*128 + u.
                # Unused needle slots hold stale floats; ignored host-side.
                nc.vector.max_index(
                    out=idxacc[:, c, :],
                    in_max=rowmax[:, :],
                    in_values=p2[:, :, :].rearrange("p a v -> p (a v)"))
                j += n
            nc.sync.dma_start(out=idx_h[:, :, :], in_=idxacc[:, :, :])

    nc.compile()
    return nc


def _get_prog():
    if "nc" not in _prog_cache:
        _prog_cache["nc"] = _build()
    return _prog_cache["nc"]


# chunk start offsets
_J0 = np.concatenate([[0], np.cumsum(CHUNKS)[:-1]]).astype(np.int64)


def run_sharded(emission: np.ndarray, **spmd_kwargs):
    """Run the SPMD kernel; returns (idx int32 [T], keep bool [T], results)."""
    emission = np.ascontiguousarray(np.asarray(emission, dtype=np.float32))
    assert emission.shape == (T_FULL, V), emission.shape
    nc = _get_prog()
    in_maps = [
        {"emission": np.ascontiguousarray(emission[c * T_SHARD:(c + 1) * T_SHARD])}
        for c in range(N_CORES)
    ]
    res = run_bass_kernel_spmd(nc, in_maps, list(range(N_CORES)), **spmd_kwargs)
    # padded [P, NCH, 8] per core -> raw [T_FULL] in timestep order
    raw = np.empty(T_FULL, dtype=np.uint32)
    for core in range(N_CORES):
        r = res.results[core]["idx_out"].reshape(P, NCH, 8)
        base = core * T_SHARD
        for c, n in enumerate(CHUNKS):
            j0 = _J0[c]
            # rows p*JPP + j0 + k for k in [0, n)
            dst = base + np.arange(P)[:, None] * JPP + j0 + np.arange(n)[None, :]
            raw[dst.ravel()] = r[:, c, 0:n].ravel()

    t_all = np.arange(T_FULL)
    j_arr = t_all % JPP
    u = (raw & np.uint32(H2 - 1)).astype(np.int64)          # slot in [0,128)
    kk = raw >> np.uint32(7)                                # row-in-chunk bits
    expected = j_arr - _J0[np.searchsorted(_J0, j_arr, side="right") - 1]
    corrupt = np.nonzero(kk != expected)[0]

    # resolve the two folded bits: candidates u, u+128, u+256, u+384
    cand = np.stack([emission[t_all, u + k * H2] for k in range(4)], axis=1)
    bits = np.argmax(cand, axis=1)                          # first max wins
    idx = (u + bits * H2).astype(np.int32)

    # cross-row FIND_INDEX8 collisions (needle matched an earlier row's
    # segment): recompute those rows exactly
    for t in corrupt:
        idx[t] = int(np.argmax(emission[t]))

    keep = np.empty(T_FULL, dtype=bool)
    keep[0] = idx[0] != 0
    keep[1:] = (idx[1:] != idx[:-1]) & (idx[1:] != 0)
    return idx, keep, res


def kernel(emission: np.ndarray):
    idx, keep, _ = run_sharded(emission)
    return idx, keep


# revision 8
# speedup vs baseline: 1.2554x; 1.0186x over previous
"""Greedy CTC decoder on Trainium2 (Bass/Tile), sharded over 8 NeuronCores.

Input : emission [65536, 512] float32 (full, unsharded)
Output: (index [65536] int32, keep [65536] bool) matching the reference:
    index = argmax(emission, axis=-1)
    keep  = (index != prev_index) & (index != 0), prev of t=0 is a sentinel

Sharding: timestep axis T split across 8 cores (8192 rows each). Inside a
core, partition p owns the 64 consecutive timesteps p*64..p*64+63.

Device pipeline per chunk of n rows (all on the DVE; TT folds read 2
elems/cycle so the total is ~1.125 data passes instead of 2):
  fold1  p1[:, n, 256] = max(x[:, :, 0:256],  x[:, :, 256:512])
  fold2  p2[:, n, 128] = max(p1[:, :, 0:128], p1[:, :, 128:256])
  fold3  p3[:, n,  64] = max(p2[:, :, 0:64],  p2[:, :, 64:128])
  rowmax[:, n] = reduce_max(p3)                  (the needles)
  FIND_INDEX8 over p3 -> raw = k*64 + u          (u = argmax slot)
The true argmax is one of {u + m*64, m=0..7}; the host resolves the three
folded bits with a vectorized 8-column compare (exact, including
first-occurrence tie-breaks) and derives the keep mask from idx.
DMA triggers alternate between the Sync and Scalar HWDGE queues so
trigger issue overlaps; find results accumulate in SBUF and are stored
with two DMAs (bulk early, tiny tail).
"""

import numpy as np

import concourse.bacc as bacc
import concourse.mybir as mybir
from concourse.tile import TileContext
from concourse.bass_utils import run_bass_kernel_spmd

N_CORES = 8
T_FULL = 65536
V = 512
H1 = V // 2                          # 256 after fold1
H2 = V // 4                          # 128 after fold2
H3 = V // 8                          # 64 after fold3
P = 128
T_SHARD = T_FULL // N_CORES          # 8192
JPP = T_SHARD // P                   # 64 timesteps per partition
CHUNKS = [1, 1, 2, 4] + [8] * 6 + [4, 2, 1, 1]
NCH = len(CHUNKS)
SPLIT_C = 10                         # chunks [0, SPLIT_C) stored early
assert sum(CHUNKS) == JPP

_prog_cache = {}


def _build():
    nc = bacc.Bacc(None, target_bir_lowering=False)

    em_h = nc.dram_tensor("emission", [T_SHARD, V], mybir.dt.float32,
                          kind="ExternalInput")
    # padded find results: slot [p, c, k] = chunk c needle k on partition p
    idx_h = nc.dram_tensor("idx_out", [P, NCH, 8], mybir.dt.uint32,
                           kind="ExternalOutput")

    em3 = em_h[:, :].rearrange("(p j) v -> p j v", p=P)

    with TileContext(nc) as tc:
        with (
            tc.tile_pool(name="io", bufs=8) as io_pool,
            tc.tile_pool(name="f1", bufs=3) as f1_pool,
            tc.tile_pool(name="f2", bufs=3) as f2_pool,
            tc.tile_pool(name="f3", bufs=3) as f3_pool,
            tc.tile_pool(name="mx", bufs=4) as mx_pool,
            tc.tile_pool(name="acc", bufs=1) as acc_pool,
        ):
            idxacc = acc_pool.tile([P, NCH, 8], mybir.dt.uint32)
            j = 0
            for c, n in enumerate(CHUNKS):
                tile = io_pool.tile([P, n, V], mybir.dt.float32)
                dma_eng = nc.sync if c % 2 == 0 else nc.scalar
                dma_eng.dma_start(out=tile[:, :, :], in_=em3[:, j:j + n, :])
                p1 = f1_pool.tile([P, n, H1], mybir.dt.float32)
                nc.vector.tensor_tensor(out=p1[:, :, :],
                                        in0=tile[:, :, 0:H1],
                                        in1=tile[:, :, H1:V],
                                        op=mybir.AluOpType.max)
                p2 = f2_pool.tile([P, n, H2], mybir.dt.float32)
                nc.vector.tensor_tensor(out=p2[:, :, :],
                                        in0=p1[:, :, 0:H2],
                                        in1=p1[:, :, H2:H1],
                                        op=mybir.AluOpType.max)
                p3 = f3_pool.tile([P, n, H3], mybir.dt.float32)
                nc.vector.tensor_tensor(out=p3[:, :, :],
                                        in0=p2[:, :, 0:H3],
                                        in1=p2[:, :, H3:H2],
                                        op=mybir.AluOpType.max)
                rowmax = mx_pool.tile([P, 8], mybir.dt.float32)
                nc.vector.tensor_reduce(out=rowmax[:, 0:n], in_=p3[:, :, :],
                                        axis=mybir.AxisListType.X,
                                        op=mybir.AluOpType.max)
                # one FIND_INDEX8 for up to 8 rows: needle k = row k's max,
                # scanned over the folded chunk; raw value = k*64 + u.
                # Unused needle slots hold stale floats; ignored host-side.
                nc.vector.max_index(
                    out=idxacc[:, c, :],
                    in_max=rowmax[:, :],
                    in_values=p3[:, :, :].rearrange("p a v -> p (a v)"))
                if c == SPLIT_C - 1:
                    nc.scalar.dma_start(out=idx_h[:, 0:SPLIT_C, :],
                                        in_=idxacc[:, 0:SPLIT_C, :])
                j += n
            nc.scalar.dma_start(out=idx_h[:, SPLIT_C:NCH, :],
                                in_=idxacc[:, SPLIT_C:NCH, :])

    nc.compile()
    return nc


def _get_prog():
    if "nc" not in _prog_cache:
        _prog_cache["nc"] = _build()
    return _prog_cache["nc"]


# chunk start offsets
_J0 = np.concatenate([[0], np.cumsum(CHUNKS)[:-1]]).astype(np.int64)


def run_sharded(emission: np.ndarray, **spmd_kwargs):
    """Run the SPMD kernel; returns (idx int32 [T], keep bool [T], results)."""
    emission = np.ascontiguousarray(np.asarray(emission, dtype=np.float32))
    assert emission.shape == (T_FULL, V), emission.shape
    nc = _get_prog()
    in_maps = [
        {"emission": np.ascontiguousarray(emission[c * T_SHARD:(c + 1) * T_SHARD])}
        for c in range(N_CORES)
    ]
    res = run_bass_kernel_spmd(nc, in_maps, list(range(N_CORES)), **spmd_kwargs)
    # padded [P, NCH, 8] per core -> raw [T_FULL] in timestep order
    raw = np.empty(T_FULL, dtype=np.uint32)
    for core in range(N_CORES):
        r = res.results[core]["idx_out"].reshape(P, NCH, 8)
        base = core * T_SHARD
        for c, n in enumerate(CHUNKS):
            j0 = _J0[c]
            dst = base + np.arange(P)[:, None] * JPP + j0 + np.arange(n)[None, :]
            raw[dst.ravel()] = r[:, c, 0:n].ravel()

    t_all = np.arange(T_FULL)
    j_arr = t_all % JPP
    u = (raw & np.uint32(H3 - 1)).astype(np.int64)          # slot in [0,64)
    kk = raw >> np.uint32(6)                                # row-in-chunk bits
    expected = j_arr - _J0[np.searchsorted(_J0, j_arr, side="right") - 1]
    corrupt = np.nonzero(kk != expected)[0]

    # resolve the three folded bits: candidates u + m*64, m = 0..7
    # (np.argmax picks the first max, matching argmax first-occurrence)
    cand = np.stack([emission[t_all, u + m * H3] for m in range(8)], axis=1)
    m_bits = np.argmax(cand, axis=1)
    idx = (u + m_bits * H3).astype(np.int32)

    # cross-row FIND_INDEX8 collisions (needle matched an earlier row's
    # segment): recompute those rows exactly
    for t in corrupt:
        idx[t] = int(np.argmax(emission[t]))

    keep = np.empty(T_FULL, dtype=bool)
    keep[0] = idx[0] != 0
    keep[1:] = (idx[1:] != idx[:-1]) & (idx[1:] != 0)
    return idx, keep, res


def kernel(emission: np.ndarray):
    idx, keep, _ = run_sharded(emission)
    return idx, keep
